# revision 2
# baseline (speedup 1.0000x reference)
"""
Trainium2 Bass kernel for batched cross-attention:
  context[b] = softmax(q[b] @ tokens[b].T / sqrt(d)) @ tokens[b]
with x_latent (tokens) [16, 4096, 768] f32, prompts_latent (q) [16, 64, 768] f32.

Sharding: data-parallel over batch - 16 batches / 8 cores = 2 per core.

Design (v6):
  - Software-pipelined phases: while group g's token tiles are PE-transposed,
    mm1 of group g-1 and pT/mm2 of group g-2 are interleaved into the same
    tensor-engine instruction stream.  The wide mm1/mm2 streams (N=512/256)
    hide the transposes' 128-col LDWEIGHTS on the weight port, and the
    steady trickle of normal-mode matmuls keeps the PE HAM clock-gate at
    8/8 (transpose-mode alone lets it drop to half clock).
  - The trailing G1 groups' [d, n] tiles are shipped pre-transposed (DMA has
    spare bandwidth vs the PE's transpose budget); they are processed last so
    the kernel tail is a short mm1->exp->pT->mm2 chain with no transposes.
  - Transposes are emitted per (batch, j-slice): work starts as soon as the
    first 394 KB j-slice of pair 0 lands.
  - Col-tiling: the two batches run concurrently in disjoint PE column halves
    (tile_position (0,0)/(0,64)).
  - Softmax row-sums come free from the exp activation's accum_out.

All operands bf16, accumulation f32.
"""

import os
import sys

import numpy as np

for _p in ("/opt/trn_rl_repo", "/root/.axon_site/_ro/trn_rl_repo"):
    if os.path.isdir(_p) and _p not in sys.path:
        sys.path.append(_p)

import ml_dtypes
from contextlib import ExitStack

import concourse.bass as bass
import concourse.mybir as mybir
import concourse.tile as tile
from concourse import bacc
from concourse.bass_utils import run_bass_kernel_spmd
from concourse.masks import make_identity

BF16 = ml_dtypes.bfloat16

N_CORES = 8
B_TOTAL = 16
BPC = B_TOTAL // N_CORES  # batches per core
N = 4096  # tokens
D = 768   # latent dim
P = 64    # prompts
DC = D // 128   # d-chunks of 128 (contraction tiles for mm1)
G = N // 512    # groups of 512 token-columns
NPAIR = G // 2  # tn is stored in pairs of groups
G1 = 2          # trailing groups whose [d, n] tiles come via DMA
NT = N // 128
SCALE = float(D) ** -0.5

_cached_nc = None


def build_bass_program() -> bass.Bass:
    nc = bacc.Bacc("TRN2", target_bir_lowering=False, debug=False)
    qt = nc.declare_dram_parameter("qt", [128, BPC, DC, P], mybir.dt.bfloat16, isOutput=False)
    tn = nc.declare_dram_parameter("tn", [NPAIR, 128, 8, BPC, D], mybir.dt.bfloat16, isOutput=False)
    if G1 > 0:
        tt = nc.declare_dram_parameter("tt", [G1, 128, BPC, DC, 512], mybir.dt.bfloat16, isOutput=False)
    out = nc.declare_dram_parameter("out", [BPC, P, D], mybir.dt.float32, isOutput=True)

    NON_SHIPPED = G - G1  # groups 0..NON_SHIPPED-1 are transposed on-chip

    with tile.TileContext(nc) as tc, ExitStack() as ctx:
        singles = ctx.enter_context(tc.tile_pool(name="singles", bufs=3))
        tn_pool = ctx.enter_context(tc.tile_pool(name="tn", bufs=4))
        tts_pool = ctx.enter_context(tc.tile_pool(name="tts", bufs=3))
        ttd_pool = ctx.enter_context(tc.tile_pool(name="ttd", bufs=max(G1, 1)))
        p_pool = ctx.enter_context(tc.tile_pool(name="pexp", bufs=3))
        pt_pool = ctx.enter_context(tc.tile_pool(name="ptT", bufs=2))
        o_pool = ctx.enter_context(tc.tile_pool(name="osb", bufs=1))
        sc_pool = ctx.enter_context(tc.tile_pool(name="scal", bufs=2))

        psum_s = ctx.enter_context(tc.tile_pool(name="psum_s", bufs=2, space="PSUM"))
        psum_tr = ctx.enter_context(tc.tile_pool(name="psum_tr", bufs=2, space="PSUM"))
        psum_pt = ctx.enter_context(tc.tile_pool(name="psum_pt", bufs=2, space="PSUM"))
        psum_o = ctx.enter_context(tc.tile_pool(name="psum_o", bufs=1, space="PSUM"))

        ident = singles.tile([128, 128], mybir.dt.bfloat16)
        make_identity(nc, ident)

        # HAM warm-up: ~3.5us of dummy matmuls while the first DMAs land flips
        # the PE clock gate to 8/8 before real work starts.
        warm = psum_s.tile([128, 512], mybir.dt.float32, name="s_ps")
        for w in range(34):
            nc.tensor.matmul(
                warm[:, (w % 4) * 128:(w % 4 + 1) * 128],
                lhsT=ident,
                rhs=ident,
                start=True,
                stop=True,
            )

        # G+1 columns: the last group's exp is split in two for tail latency
        sums_t = singles.tile([128, G + 1], mybir.dt.float32, name="sums_t")

        o_ab = [None]
        tn_tiles = {}   # pair -> tile [128, 8, BPC, D]
        ttd_tiles = {}  # g -> tile [128, BPC, DC, 512]
        tts_tiles = {}  # g -> tile [128, BPC, DC, 512] (on-chip transposed)
        p_tiles = {}    # g -> p_sb tile
        pt_tiles = {}   # g -> pt_sb tile

        def load_pair(p, eng, split=0):
            t = tn_pool.tile([128, 8, BPC, D], mybir.dt.bfloat16, name="tn_p")
            if split:
                step = 8 // split
                for h in range(split):
                    eng.dma_start(
                        out=t[:, h * step:(h + 1) * step],
                        in_=tn[p, :, h * step:(h + 1) * step],
                    )
            else:
                eng.dma_start(out=t, in_=tn[p])
            tn_tiles[p] = t

        def load_ttd(g, eng):
            gg = g - (G - G1)
            t = ttd_pool.tile([128, BPC, DC, 512], mybir.dt.bfloat16, name="tt_d")
            eng.dma_start(out=t, in_=tt[gg])
            ttd_tiles[g] = t

        # copy-engine rotation for the PSUM->SBUF transpose copies
        cp_k = [0]

        def rot_copy(dst, src):
            k = cp_k[0]
            cp_k[0] += 1
            e = (nc.vector, nc.vector, nc.scalar)[k % 3]
            if e is nc.scalar:
                e.copy(dst, src)
            else:
                e.tensor_copy(dst, src)

        def tr_slice(g, b, j):
            """Transpose the 6 c-chunks of (group g, batch b, j-slice j):
            6 PE transposes into one PSUM tile, then one copy into tts(g)."""
            if g not in tts_tiles:
                tts_tiles[g] = tts_pool.tile(
                    [128, BPC, DC, 512], mybir.dt.bfloat16, name="tts"
                )
            tn_p = tn_tiles[g // 2]
            jj0 = (g % 2) * 4
            tr = psum_tr.tile([128, DC, 128], mybir.dt.bfloat16, name="tr_ps")
            for c in range(DC):
                nc.tensor.transpose(
                    tr[:, c, :],
                    tn_p[:, jj0 + j, b, c * 128:(c + 1) * 128],
                    ident,
                )
            rot_copy(tts_tiles[g][:, b, :, j * 128:(j + 1) * 128], tr)

        def mm1_chunk(g, c):
            if g not in mm1_psum:
                mm1_psum[g] = psum_s.tile([128, 512], mybir.dt.float32, name="s_ps")
            s_ps = mm1_psum[g]
            src = ttd_tiles.get(g)
            if src is None:
                src = tts_tiles[g]
            for b in range(BPC):
                nc.tensor.matmul(
                    s_ps[b * P:(b + 1) * P, :],
                    lhsT=qt_t[:, b, c, :],
                    rhs=src[:, b, c, :],
                    start=(c == 0),
                    stop=(c == DC - 1),
                    tile_position=(0, b * P),
                )

        mm1_psum = {}

        def softmax(g, split=False):
            s_ps = mm1_psum.pop(g)
            p_sb = p_pool.tile([128, 512], mybir.dt.bfloat16, name="p_sb")
            if not split:
                nc.scalar.activation(
                    out=p_sb,
                    in_=s_ps,
                    func=mybir.ActivationFunctionType.Exp,
                    scale=SCALE,
                    accum_out=sums_t[:, g:g + 1],
                )
            else:
                for h in range(2):
                    nc.scalar.activation(
                        out=p_sb[:, h * 256:(h + 1) * 256],
                        in_=s_ps[:, h * 256:(h + 1) * 256],
                        func=mybir.ActivationFunctionType.Exp,
                        scale=SCALE,
                        accum_out=sums_t[:, g + h:g + h + 1],
                    )
            p_tiles[g] = p_sb

        def p_transpose(g, js=(0, 1, 2, 3)):
            p_sb = p_tiles[g]
            if g not in pt_tiles:
                pt_tiles[g] = pt_pool.tile([128, 4, 128], mybir.dt.bfloat16, name="pt_sb")
            pt_ps = psum_pt.tile([128, len(js), 128], mybir.dt.bfloat16, name="pt_ps")
            for k, j in enumerate(js):
                nc.tensor.transpose(
                    pt_ps[:, k, :], p_sb[:, j * 128:(j + 1) * 128], ident
                )
            nc.vector.tensor_copy(
                pt_tiles[g][:, js[0]:js[0] + len(js), :], pt_ps
            )

        def mm2_j(g, j):
            if o_ab[0] is None:
                o_a = psum_o.tile([128, 512], mybir.dt.float32, tag="o_a")
                o_b = psum_o.tile([128, 256], mybir.dt.float32, tag="o_b")
                o_ab[0] = (o_a, o_b)
            o_a, o_b = o_ab[0]
            tn_p = tn_tiles[g // 2]
            jj0 = (g % 2) * 4
            pt_sb = pt_tiles[g]
            nt = g * 4 + j
            for b in range(BPC):
                nc.tensor.matmul(
                    o_a[b * P:(b + 1) * P, :],
                    lhsT=pt_sb[:, j, b * P:(b + 1) * P],
                    rhs=tn_p[:, jj0 + j, b, 0:512],
                    start=(nt == 0),
                    stop=(nt == NT - 1),
                    tile_position=(0, b * P),
                )
                nc.tensor.matmul(
                    o_b[b * P:(b + 1) * P, :],
                    lhsT=pt_sb[:, j, b * P:(b + 1) * P],
                    rhs=tn_p[:, jj0 + j, b, 512:768],
                    start=(nt == 0),
                    stop=(nt == NT - 1),
                    tile_position=(0, b * P),
                )

        def finish():
            tot = sc_pool.tile([128, 1], mybir.dt.float32, name="tot")
            nc.vector.reduce_sum(tot, sums_t, axis=mybir.AxisListType.X)
            rec = sc_pool.tile([128, 1], mybir.dt.float32, name="rec")
            nc.vector.reciprocal(rec, tot)
            o_a, o_b = o_ab[0]
            o_sb = o_pool.tile([128, D], mybir.dt.float32, name="o_sb")
            # normalize the two PSUM slabs on different engines, store each
            # half as soon as it is ready (one store per HWDGE ring)
            nc.vector.tensor_scalar_mul(o_sb[:, 0:512], o_a, rec)
            nc.scalar.mul(o_sb[:, 512:768], o_b, rec)
            nc.sync.dma_start(out=out[0], in_=o_sb[0:P, :])
            nc.scalar.dma_start(out=out[1], in_=o_sb[P:2 * P, :])

        # ---- DMA schedule ----
        # scalar ring: qt (tiny, lands first), later the output stores.
        # sync ring: the ordered token stream - pair0 j-sliced for the
        # earliest possible transpose start, then pair1/pair2, then the
        # two shipped [d,n] groups, then pair3 (natural layout, only needed
        # by mm2 of the shipped groups at the very end).
        qt_t = singles.tile([128, BPC, DC, P], mybir.dt.bfloat16, name="qt_t")
        nc.scalar.dma_start(out=qt_t, in_=qt[:])
        load_pair(0, nc.sync, split=8)
        load_pair(1, nc.sync, split=2)
        load_pair(2, nc.sync, split=2)
        if G1 >= 1:
            load_ttd(G - G1, nc.sync)
        if G1 >= 2:
            load_ttd(G - G1 + 1, nc.sync)
        for k in range(2, G1):
            load_ttd(G - G1 + k, nc.sync)
        load_pair(3, nc.sync, split=2)

        # ---- phase-pipelined emission ----
        # Phase t transposes group t (if on-chip), runs mm1 of group t-1,
        # and pT + mm2 of group t-2, all interleaved on the tensor queue.
        # Shipped groups need no transpose phase, so the schedule naturally
        # drains with two short mm1-only / mm2-only phases.
        def phase_work(t):
            """Returns (tr_items, mm1_items, tail_items) for phase t."""
            trs = []
            if t < NON_SHIPPED:
                for j in range(4):
                    for b in range(BPC):
                        trs.append((t, b, j))
            mm1s = []
            g1g = t - 1
            if 0 <= g1g < G:
                for c in range(DC):
                    mm1s.append((g1g, c))
            tails = []
            g2g = t - 2
            if 0 <= g2g < G:
                tails.append(("pt", g2g))
                for j in range(4):
                    tails.append(("mm2", g2g, j))
            return trs, mm1s, tails

        for t in range(G + 2):
            trs, mm1s, tails = phase_work(t)
            # interleave: round-robin the three streams so transposes hide
            # their LDWEIGHTS under the wide mm1/mm2 streams
            items = []
            ntr, nm, ntl = len(trs), len(mm1s), len(tails)
            steps = max(ntr, nm, ntl, 1)
            ti = mi = li = 0
            for s in range(steps):
                hi_tr = (s + 1) * ntr // steps
                hi_m = (s + 1) * nm // steps
                hi_l = (s + 1) * ntl // steps
                while ti < hi_tr:
                    items.append(("tr", trs[ti])); ti += 1
                while mi < hi_m:
                    items.append(("mm1", mm1s[mi])); mi += 1
                while li < hi_l:
                    items.append(("tail", tails[li])); li += 1
            for kind, it in items:
                if kind == "tr":
                    g, b, j = it
                    tr_slice(g, b, j)
                elif kind == "mm1":
                    g, c = it
                    mm1_chunk(g, c)
                    if c == DC - 1:
                        softmax(g, split=(g == G - 1))
                else:
                    if it[0] == "pt":
                        p_transpose(it[1])
                    else:
                        _, g, j = it
                        mm2_j(g, j)
        finish()

    nc.compile()
    return nc


def _get_nc() -> bass.Bass:
    global _cached_nc
    if _cached_nc is None:
        _cached_nc = build_bass_program()
    return _cached_nc


def _make_in_maps(x_latent: np.ndarray, prompts_latent: np.ndarray):
    x8 = np.ascontiguousarray(x_latent.astype(BF16)).reshape(N_CORES, BPC, N, D)
    q8 = prompts_latent.astype(BF16).reshape(N_CORES, BPC, P, D)
    # tn: [core, NPAIR, 128, 8, BPC, D] - j-major so split loads slice
    # contiguous per-partition runs
    tn_sw = np.ascontiguousarray(
        x8.reshape(N_CORES, BPC, NPAIR, 8, 128, D).transpose(0, 2, 4, 3, 1, 5)
    )
    # qt: [core, 128, BPC, DC, P]
    qt_sw = np.ascontiguousarray(
        q8.transpose(0, 1, 3, 2).reshape(N_CORES, BPC, DC, 128, P).transpose(0, 3, 1, 2, 4)
    )
    maps = []
    if G1 > 0:
        # tt: [core, G1, 128, BPC, DC, 512]
        ttf = x8.transpose(0, 1, 3, 2)                      # [core, b, D, N]
        arr = ttf.reshape(N_CORES, BPC, DC, 128, G, 512)
        tt_sw = np.ascontiguousarray(
            arr[:, :, :, :, G - G1:, :].transpose(0, 4, 3, 1, 2, 5)
        )
    for c in range(N_CORES):
        m = {"qt": qt_sw[c], "tn": tn_sw[c]}
        if G1 > 0:
            m["tt"] = tt_sw[c]
        maps.append(m)
    return maps


def run(x_latent: np.ndarray, prompts_latent: np.ndarray, trace: bool = False):
    """Run on all 8 cores; returns (output [16, 64, 768] f32, BassKernelResults)."""
    nc = _get_nc()
    in_maps = _make_in_maps(np.asarray(x_latent), np.asarray(prompts_latent))
    res = run_bass_kernel_spmd(nc, in_maps, list(range(N_CORES)), trace=trace)
    out = np.concatenate([np.asarray(r["out"]) for r in res.results], axis=0)
    return out.astype(np.float32), res


def kernel(x_latent: np.ndarray, prompts_latent: np.ndarray) -> np.ndarray:
    out, _ = run(x_latent, prompts_latent, trace=False)
    return out


# revision 3
# speedup vs baseline: 1.1034x; 1.1034x over previous
"""
Trainium2 Bass kernel for batched cross-attention:
  context[b] = softmax(q[b] @ tokens[b].T / sqrt(d)) @ tokens[b]
with x_latent (tokens) [16, 4096, 768] f32, prompts_latent (q) [16, 64, 768] f32.

Sharding: data-parallel over batch - 16 batches / 8 cores = 2 per core.

Design (v7):
  - Single-phase pipeline: group g's transposes AND its mm1 run in the same
    phase, with mm1 emitted j-sliced (N=128 chunks) one j-slice behind the
    transposes.  pT+mm2 of group g-1 are slotted between.  Every PE
    instruction's off-engine dependencies (PSUM->SBUF copies, exp) complete
    well before the strict-FIFO tensor queue reaches it, and the steady
    trickle of normal-mode matmuls keeps the HAM clock-gate at 8/8.
  - The transposes' 128-col LDWEIGHTS hide under the mm1/mm2 streams on the
    weight port.
  - The trailing G1 groups' [d, n] tiles are shipped pre-transposed and
    processed last: the kernel tail is a short mm1->exp->pT->mm2 chain,
    split in column halves to overlap exp with pT/mm2.
  - All PSUM->SBUF transpose copies ride the vector engine (fast, 530ns);
    scalar only does exp/accum plus the o_b normalize.

All operands bf16, accumulation f32.
"""

import os
import sys

import numpy as np

for _p in ("/opt/trn_rl_repo", "/root/.axon_site/_ro/trn_rl_repo"):
    if os.path.isdir(_p) and _p not in sys.path:
        sys.path.append(_p)

import ml_dtypes
from contextlib import ExitStack

import concourse.bass as bass
import concourse.mybir as mybir
import concourse.tile as tile
from concourse import bacc
from concourse.bass_utils import run_bass_kernel_spmd
from concourse.masks import make_identity

BF16 = ml_dtypes.bfloat16

N_CORES = 8
B_TOTAL = 16
BPC = B_TOTAL // N_CORES  # batches per core
N = 4096  # tokens
D = 768   # latent dim
P = 64    # prompts
DC = D // 128   # d-chunks of 128 (contraction tiles for mm1)
G = N // 512    # groups of 512 token-columns
NPAIR = G // 2  # tn is stored in pairs of groups
G1 = 2          # trailing groups whose [d, n] tiles come via DMA
NT = N // 128
SCALE = float(D) ** -0.5

_cached_nc = None


def build_bass_program() -> bass.Bass:
    nc = bacc.Bacc("TRN2", target_bir_lowering=False, debug=False)
    qt = nc.declare_dram_parameter("qt", [128, BPC, DC, P], mybir.dt.bfloat16, isOutput=False)
    tn = nc.declare_dram_parameter("tn", [NPAIR, 128, 8, BPC, D], mybir.dt.bfloat16, isOutput=False)
    if G1 > 0:
        tt = nc.declare_dram_parameter("tt", [G1, 128, BPC, DC, 512], mybir.dt.bfloat16, isOutput=False)
    out = nc.declare_dram_parameter("out", [BPC, P, D], mybir.dt.float32, isOutput=True)

    NS = G - G1  # groups 0..NS-1 are transposed on-chip

    with tile.TileContext(nc) as tc, ExitStack() as ctx:
        singles = ctx.enter_context(tc.tile_pool(name="singles", bufs=3))
        tn_pool = ctx.enter_context(tc.tile_pool(name="tn", bufs=4))
        tts_pool = ctx.enter_context(tc.tile_pool(name="tts", bufs=2))
        ttd_pool = ctx.enter_context(tc.tile_pool(name="ttd", bufs=max(G1, 1)))
        p_pool = ctx.enter_context(tc.tile_pool(name="pexp", bufs=3))
        pt_pool = ctx.enter_context(tc.tile_pool(name="ptT", bufs=2))
        o_pool = ctx.enter_context(tc.tile_pool(name="osb", bufs=1))
        sc_pool = ctx.enter_context(tc.tile_pool(name="scal", bufs=2))

        psum_s = ctx.enter_context(tc.tile_pool(name="psum_s", bufs=2, space="PSUM"))
        psum_tr = ctx.enter_context(tc.tile_pool(name="psum_tr", bufs=3, space="PSUM"))
        psum_pt = ctx.enter_context(tc.tile_pool(name="psum_pt", bufs=1, space="PSUM"))
        psum_o = ctx.enter_context(tc.tile_pool(name="psum_o", bufs=1, space="PSUM"))

        ident = singles.tile([128, 128], mybir.dt.bfloat16)
        make_identity(nc, ident)

        # HAM warm-up: ~5us of dummy matmuls while the first DMAs land flips
        # the PE clock gate to 8/8 before real work starts.
        warm = psum_s.tile([128, 512], mybir.dt.float32, name="s_ps")
        for w in range(48):
            nc.tensor.matmul(
                warm[:, (w % 4) * 128:(w % 4 + 1) * 128],
                lhsT=ident,
                rhs=ident,
                start=True,
                stop=True,
            )

        # G+1 columns: the last group's exp is split in two for tail latency
        sums_t = singles.tile([128, G + 1], mybir.dt.float32, name="sums_t")

        o_ab = [None]
        tn_tiles = {}   # pair -> tile [128, 8, BPC, D]
        ttd_tiles = {}  # g -> tile [128, BPC, DC, 512]
        tts_tiles = {}  # g -> tile [128, BPC, DC, 512] (on-chip transposed)
        p_tiles = {}    # g -> p_sb tile
        pt_tiles = {}   # g -> pt_sb tile
        mm1_psum = {}   # g -> s_ps tile

        def load_pair(p, eng, split=0):
            t = tn_pool.tile([128, 8, BPC, D], mybir.dt.bfloat16, name="tn_p")
            if split:
                step = 8 // split
                for h in range(split):
                    eng.dma_start(
                        out=t[:, h * step:(h + 1) * step],
                        in_=tn[p, :, h * step:(h + 1) * step],
                    )
            else:
                eng.dma_start(out=t, in_=tn[p])
            tn_tiles[p] = t

        def load_ttd(g, eng):
            gg = g - (G - G1)
            t = ttd_pool.tile([128, BPC, DC, 512], mybir.dt.bfloat16, name="tt_d")
            eng.dma_start(out=t, in_=tt[gg])
            ttd_tiles[g] = t

        def tr_slice(g, b, j):
            """Transpose the 6 c-chunks of (group g, batch b, j-slice j):
            6 PE transposes into one PSUM tile, then one DVE copy into tts(g)."""
            if g not in tts_tiles:
                tts_tiles[g] = tts_pool.tile(
                    [128, BPC, DC, 512], mybir.dt.bfloat16, name="tts"
                )
            tn_p = tn_tiles[g // 2]
            jj0 = (g % 2) * 4
            tr = psum_tr.tile([128, DC, 128], mybir.dt.bfloat16, name="tr_ps")
            for c in range(DC):
                nc.tensor.transpose(
                    tr[:, c, :],
                    tn_p[:, jj0 + j, b, c * 128:(c + 1) * 128],
                    ident,
                )
            nc.vector.tensor_copy(
                tts_tiles[g][:, b, :, j * 128:(j + 1) * 128], tr
            )

        def mm1_j(g, j):
            """mm1 of group g restricted to n-columns [j*128, (j+1)*128):
            12 N=128 matmuls (2 batches col-tiled x 6 c-chunks)."""
            if g not in mm1_psum:
                mm1_psum[g] = psum_s.tile([128, 512], mybir.dt.float32, name="s_ps")
            s_ps = mm1_psum[g]
            src = tts_tiles[g]
            for c in range(DC):
                for b in range(BPC):
                    nc.tensor.matmul(
                        s_ps[b * P:(b + 1) * P, j * 128:(j + 1) * 128],
                        lhsT=qt_t[:, b, c, :],
                        rhs=src[:, b, c, j * 128:(j + 1) * 128],
                        start=(c == 0),
                        stop=(c == DC - 1),
                        tile_position=(0, b * P),
                    )

        def mm1_chunk(g, c):
            """mm1 c-chunk for shipped groups (full 512-wide stream)."""
            if g not in mm1_psum:
                mm1_psum[g] = psum_s.tile([128, 512], mybir.dt.float32, name="s_ps")
            s_ps = mm1_psum[g]
            src = ttd_tiles[g]
            for b in range(BPC):
                nc.tensor.matmul(
                    s_ps[b * P:(b + 1) * P, :],
                    lhsT=qt_t[:, b, c, :],
                    rhs=src[:, b, c, :],
                    start=(c == 0),
                    stop=(c == DC - 1),
                    tile_position=(0, b * P),
                )

        def exp_full(g):
            s_ps = mm1_psum.pop(g)
            p_sb = p_pool.tile([128, 512], mybir.dt.bfloat16, name="p_sb")
            nc.scalar.activation(
                out=p_sb,
                in_=s_ps,
                func=mybir.ActivationFunctionType.Exp,
                scale=SCALE,
                accum_out=sums_t[:, g:g + 1],
            )
            p_tiles[g] = p_sb

        def exp_half(g, h):
            s_ps = mm1_psum[g]
            if g not in p_tiles:
                p_tiles[g] = p_pool.tile([128, 512], mybir.dt.bfloat16, name="p_sb")
            nc.scalar.activation(
                out=p_tiles[g][:, h * 256:(h + 1) * 256],
                in_=s_ps[:, h * 256:(h + 1) * 256],
                func=mybir.ActivationFunctionType.Exp,
                scale=SCALE,
                accum_out=sums_t[:, g + h:g + h + 1],
            )

        def p_transpose(g, js=(0, 1, 2, 3)):
            p_sb = p_tiles[g]
            if g not in pt_tiles:
                pt_tiles[g] = pt_pool.tile([128, 4, 128], mybir.dt.bfloat16, name="pt_sb")
            pt_ps = psum_pt.tile([128, len(js), 128], mybir.dt.bfloat16, name="pt_ps")
            for k, j in enumerate(js):
                nc.tensor.transpose(
                    pt_ps[:, k, :], p_sb[:, j * 128:(j + 1) * 128], ident
                )
            nc.vector.tensor_copy(
                pt_tiles[g][:, js[0]:js[0] + len(js), :], pt_ps
            )

        def mm2_j(g, j):
            if o_ab[0] is None:
                o_a = psum_o.tile([128, 512], mybir.dt.float32, tag="o_a")
                o_b = psum_o.tile([128, 256], mybir.dt.float32, tag="o_b")
                o_ab[0] = (o_a, o_b)
            o_a, o_b = o_ab[0]
            tn_p = tn_tiles[g // 2]
            jj0 = (g % 2) * 4
            pt_sb = pt_tiles[g]
            nt = g * 4 + j
            for b in range(BPC):
                nc.tensor.matmul(
                    o_a[b * P:(b + 1) * P, :],
                    lhsT=pt_sb[:, j, b * P:(b + 1) * P],
                    rhs=tn_p[:, jj0 + j, b, 0:512],
                    start=(nt == 0),
                    stop=(nt == NT - 1),
                    tile_position=(0, b * P),
                )
                nc.tensor.matmul(
                    o_b[b * P:(b + 1) * P, :],
                    lhsT=pt_sb[:, j, b * P:(b + 1) * P],
                    rhs=tn_p[:, jj0 + j, b, 512:768],
                    start=(nt == 0),
                    stop=(nt == NT - 1),
                    tile_position=(0, b * P),
                )

        def finish():
            tot = sc_pool.tile([128, 1], mybir.dt.float32, name="tot")
            nc.vector.reduce_sum(tot, sums_t, axis=mybir.AxisListType.X)
            rec = sc_pool.tile([128, 1], mybir.dt.float32, name="rec")
            nc.vector.reciprocal(rec, tot)
            o_a, o_b = o_ab[0]
            o_sb = o_pool.tile([128, D], mybir.dt.float32, name="o_sb")
            nc.vector.tensor_scalar_mul(o_sb[:, 0:512], o_a, rec)
            nc.scalar.mul(o_sb[:, 512:768], o_b, rec)
            nc.sync.dma_start(out=out[0], in_=o_sb[0:P, :])
            nc.scalar.dma_start(out=out[1], in_=o_sb[P:2 * P, :])

        # ---- DMA schedule ----
        qt_t = singles.tile([128, BPC, DC, P], mybir.dt.bfloat16, name="qt_t")
        nc.scalar.dma_start(out=qt_t, in_=qt[:])
        load_pair(0, nc.sync, split=4)
        load_pair(1, nc.sync, split=2)
        load_pair(2, nc.sync, split=2)
        if G1 >= 1:
            load_ttd(G - G1, nc.sync)
        if G1 >= 2:
            load_ttd(G - G1 + 1, nc.sync)
        for k in range(2, G1):
            load_ttd(G - G1 + k, nc.sync)
        load_pair(3, nc.sync, split=2)

        # ---- phase-pipelined emission ----
        # Phase t (t < NS):  tr(t) + j-sliced mm1(t), with pT(t-1)+mm2(t-1)
        # slotted in; exp(t) at the end.
        # Phase t in [NS, G): shipped group t - mm1 chunks + pT/mm2 of t-1.
        # Phase G: split tail of group G-1, then finish.
        for t in range(G):
            g_pre = t - 1
            if t < NS:
                # emission order chosen so that every consumer is reached by
                # the PE queue well after its producer (copy/exp) completed
                tr_slice(t, 0, 0)
                tr_slice(t, 1, 0)
                if g_pre >= 0:
                    p_transpose(g_pre)
                tr_slice(t, 0, 1)
                tr_slice(t, 1, 1)
                mm1_j(t, 0)
                if g_pre >= 0:
                    mm2_j(g_pre, 0)
                tr_slice(t, 0, 2)
                tr_slice(t, 1, 2)
                mm1_j(t, 1)
                if g_pre >= 0:
                    mm2_j(g_pre, 1)
                tr_slice(t, 0, 3)
                tr_slice(t, 1, 3)
                mm1_j(t, 2)
                if g_pre >= 0:
                    mm2_j(g_pre, 2)
                mm1_j(t, 3)
                if g_pre >= 0:
                    mm2_j(g_pre, 3)
                exp_full(t)
            else:
                # shipped group: mm1 chunks interleaved with prev group's tail
                p_transpose(g_pre)
                mm1_chunk(t, 0)
                mm1_chunk(t, 1)
                mm2_j(g_pre, 0)
                mm1_chunk(t, 2)
                mm2_j(g_pre, 1)
                mm1_chunk(t, 3)
                mm2_j(g_pre, 2)
                mm1_chunk(t, 4)
                mm2_j(g_pre, 3)
                mm1_chunk(t, 5)
                if t < G - 1:
                    exp_full(t)
                else:
                    exp_half(t, 0)

        # tail of the last group, split in column halves:
        # half 0's pT/mm2 overlap half 1's exp
        g = G - 1
        exp_half(g, 1)  # emitted after half 0; ACT runs them back-to-back
        p_transpose(g, js=(0, 1))
        mm2_j(g, 0)
        p_transpose(g, js=(2, 3))
        mm2_j(g, 1)
        mm2_j(g, 2)
        mm2_j(g, 3)
        finish()

    nc.compile()
    return nc


def _get_nc() -> bass.Bass:
    global _cached_nc
    if _cached_nc is None:
        _cached_nc = build_bass_program()
    return _cached_nc


def _make_in_maps(x_latent: np.ndarray, prompts_latent: np.ndarray):
    x8 = np.ascontiguousarray(x_latent.astype(BF16)).reshape(N_CORES, BPC, N, D)
    q8 = prompts_latent.astype(BF16).reshape(N_CORES, BPC, P, D)
    # tn: [core, NPAIR, 128, 8, BPC, D] - j-major so split loads slice
    # contiguous per-partition runs
    tn_sw = np.ascontiguousarray(
        x8.reshape(N_CORES, BPC, NPAIR, 8, 128, D).transpose(0, 2, 4, 3, 1, 5)
    )
    # qt: [core, 128, BPC, DC, P]
    qt_sw = np.ascontiguousarray(
        q8.transpose(0, 1, 3, 2).reshape(N_CORES, BPC, DC, 128, P).transpose(0, 3, 1, 2, 4)
    )
    maps = []
    if G1 > 0:
        # tt: [core, G1, 128, BPC, DC, 512]
        ttf = x8.transpose(0, 1, 3, 2)                      # [core, b, D, N]
        arr = ttf.reshape(N_CORES, BPC, DC, 128, G, 512)
        tt_sw = np.ascontiguousarray(
            arr[:, :, :, :, G - G1:, :].transpose(0, 4, 3, 1, 2, 5)
        )
    for c in range(N_CORES):
        m = {"qt": qt_sw[c], "tn": tn_sw[c]}
        if G1 > 0:
            m["tt"] = tt_sw[c]
        maps.append(m)
    return maps


def run(x_latent: np.ndarray, prompts_latent: np.ndarray, trace: bool = False):
    """Run on all 8 cores; returns (output [16, 64, 768] f32, BassKernelResults)."""
    nc = _get_nc()
    in_maps = _make_in_maps(np.asarray(x_latent), np.asarray(prompts_latent))
    res = run_bass_kernel_spmd(nc, in_maps, list(range(N_CORES)), trace=trace)
    out = np.concatenate([np.asarray(r["out"]) for r in res.results], axis=0)
    return out.astype(np.float32), res


def kernel(x_latent: np.ndarray, prompts_latent: np.ndarray) -> np.ndarray:
    out, _ = run(x_latent, prompts_latent, trace=False)
    return out
